# revision 5
# baseline (speedup 1.0000x reference)
"""Trainium2 Bass kernel for nn_Loop_Projection (batched per-prototype GEMM).

Computes out[b, e, p] = sum_d x[b, d, p] * W[p, d, e] + b[p, e] with
x: [256, 512, 128] f32, W: [128, 512, 128] f32, b: [128, 128] f32.

Sharding: prototype axis P=128 split across 8 NeuronCores (16 protos each).
Each core's x/W slices are pre-transposed on the host so that every device
DMA is fully contiguous:
  xk[p][k, c*B + b] = x[b, 128c + k, p]      ([16, 128, 1024] per core)
  wk[p][k, c*E + e] = W[p, 128c + k, e]      ([16, 128, 512]  per core)
Per proto the kernel accumulates out.T = W_p.T @ x_p.T ([E, B] PSUM tile)
over 4 K-chunks of 128, adds the bias during the PSUM->SBUF copy, and
stores y[p] = [E, B] contiguous. The host reassembles [B, E, P].
"""

import os

import numpy as np

import concourse.bass as bass
import concourse.tile as tile
from concourse import bacc, mybir
from concourse.bass_utils import run_bass_kernel_spmd

B, D, P, E = 256, 512, 128, 128
NCORES = 8
PL = P // NCORES  # prototypes per core
KC = D // 128  # contraction chunks of 128

_nc_cache = None
LAST_RESULTS = None  # BassKernelResults of the most recent run (for test.py)


def _build_nc() -> bass.Bass:
    nc = bacc.Bacc()
    xk = nc.dram_tensor("xk", [PL, 128, KC * B], mybir.dt.float32, kind="ExternalInput")
    wk = nc.dram_tensor("wk", [PL, 128, KC * E], mybir.dt.float32, kind="ExternalInput")
    bT = nc.dram_tensor("bT", [E, PL], mybir.dt.float32, kind="ExternalInput")
    y = nc.dram_tensor("y", [PL, E, B], mybir.dt.float32, kind="ExternalOutput")

    with tile.TileContext(nc) as tc:
        with (
            tc.tile_pool(name="const", bufs=1) as cpool,
            tc.tile_pool(name="xin", bufs=3) as xpool,
            tc.tile_pool(name="win", bufs=3) as wpool,
            tc.tile_pool(name="acc", bufs=4, space="PSUM") as ppool,
            tc.tile_pool(name="out", bufs=4) as opool,
        ):
            bt = cpool.tile([E, PL], mybir.dt.float32)
            nc.sync.dma_start(bt[:], bT[:])
            for p in range(PL):
                xt = xpool.tile([128, KC * B], mybir.dt.float32)
                nc.sync.dma_start(xt[:], xk[p])
                wt = wpool.tile([128, KC * E], mybir.dt.float32)
                nc.sync.dma_start(wt[:], wk[p])
                ps = ppool.tile([E, B], mybir.dt.float32)
                for c in range(KC):
                    nc.tensor.matmul(
                        ps[:],
                        lhsT=wt[:, c * E : (c + 1) * E],
                        rhs=xt[:, c * B : (c + 1) * B],
                        start=(c == 0),
                        stop=(c == KC - 1),
                    )
                ot = opool.tile([E, B], mybir.dt.float32)
                nc.scalar.activation(
                    ot[:],
                    ps[:],
                    mybir.ActivationFunctionType.Identity,
                    bias=bt[:, p : p + 1],
                )
                nc.sync.dma_start(y[p], ot[:])
    nc.compile()
    return nc


def _shard_inputs(x: np.ndarray, W: np.ndarray, b: np.ndarray):
    # xk[p, k, c*B + b] = x[b, 128c + k, p]
    xk = (
        x.transpose(2, 1, 0)
        .reshape(P, KC, 128, B)
        .transpose(0, 2, 1, 3)
        .reshape(P, 128, KC * B)
    )
    # wk[p, k, c*E + e] = W[p, 128c + k, e]
    wk = W.reshape(P, KC, 128, E).transpose(0, 2, 1, 3).reshape(P, 128, KC * E)
    bT = b.T  # [E, P]
    in_maps = []
    for m in range(NCORES):
        sl = slice(m * PL, (m + 1) * PL)
        in_maps.append(
            {
                "xk": np.ascontiguousarray(xk[sl]),
                "wk": np.ascontiguousarray(wk[sl]),
                "bT": np.ascontiguousarray(bT[:, sl]),
            }
        )
    return in_maps


def kernel(x: np.ndarray, W: np.ndarray, b: np.ndarray) -> np.ndarray:
    global _nc_cache, LAST_RESULTS
    x = np.ascontiguousarray(np.asarray(x, dtype=np.float32))
    W = np.ascontiguousarray(np.asarray(W, dtype=np.float32))
    b = np.ascontiguousarray(np.asarray(b, dtype=np.float32))
    if _nc_cache is None:
        _nc_cache = _build_nc()
    in_maps = _shard_inputs(x, W, b)
    res = run_bass_kernel_spmd(
        _nc_cache,
        in_maps,
        core_ids=list(range(NCORES)),
        trace=bool(os.environ.get("KERNEL_TRACE")),
    )
    LAST_RESULTS = res
    yall = np.concatenate([r["y"] for r in res.results], axis=0)  # [P, E, B]
    return np.ascontiguousarray(yall.transpose(2, 1, 0))  # [B, E, P]


# revision 7
# speedup vs baseline: 1.1000x; 1.1000x over previous
"""Trainium2 Bass kernel for nn_Loop_Projection (batched per-prototype GEMM).

Computes out[b, e, p] = sum_d x[b, d, p] * W[p, d, e] + b[p, e] with
x: [256, 512, 128] f32, W: [128, 512, 128] f32, b: [128, 128] f32.

Sharding: prototype axis P=128 split across 8 NeuronCores (16 protos each).
Each core's x/W slices are pre-transposed on the host so that every device
DMA is fully contiguous:
  xk[p][k, c*B + b] = x[b, 128c + k, p]      ([16, 128, 1024] per core)
  wk[p][k, c*E + e] = W[p, 128c + k, e]      ([16, 128, 512]  per core)
Per proto the kernel accumulates out.T = W_p.T @ x_p.T ([E, B] PSUM tile)
over 4 K-chunks of 128, adds the bias during the PSUM->SBUF copy, and
stores y[p] = [E, B] contiguous. The host reassembles [B, E, P].
"""

import os

import numpy as np

import concourse.bass as bass
import concourse.tile as tile
from concourse import bacc, mybir
from concourse.bass_utils import run_bass_kernel_spmd

B, D, P, E = 256, 512, 128, 128
NCORES = 8
PL = P // NCORES  # prototypes per core
KC = D // 128  # contraction chunks of 128

_nc_cache = None
LAST_RESULTS = None  # BassKernelResults of the most recent run (for test.py)

USE_FP32R = True  # float32r matmul: 1 cycle/row vs 4 for float32


def _build_nc() -> bass.Bass:
    nc = bacc.Bacc()
    xk = nc.dram_tensor("xk", [PL, 128, KC * B], mybir.dt.float32, kind="ExternalInput")
    wk = nc.dram_tensor("wk", [PL, 128, KC * E], mybir.dt.float32, kind="ExternalInput")
    bT = nc.dram_tensor("bT", [E, PL], mybir.dt.float32, kind="ExternalInput")
    y = nc.dram_tensor("y", [PL, E, B], mybir.dt.float32, kind="ExternalOutput")

    mm_dt = mybir.dt.float32r if USE_FP32R else mybir.dt.float32
    XW = KC * B  # 1024, x tile free width
    with tile.TileContext(nc) as tc:
        with (
            tc.tile_pool(name="const", bufs=1) as cpool,
            tc.tile_pool(name="xin", bufs=4) as xpool,
            tc.tile_pool(name="win", bufs=4) as wpool,
            tc.tile_pool(name="acc", bufs=4, space="PSUM") as ppool,
            tc.tile_pool(name="out", bufs=4) as opool,
        ):
            bt = cpool.tile([E, PL], mybir.dt.float32)
            nc.scalar.dma_start(bt[:], bT[:])
            for p in range(PL):
                # Split each x load across both HWDGE rings (SP + Act) and
                # alternate the W loads so both rings carry ~6 MiB; stores
                # ride the gpsimd SWDGE ring. One ring alone caps at ~260
                # GB/s, below the ~358 GB/s HBM-per-core limit.
                xt = xpool.tile([128, XW], mm_dt)
                nc.sync.dma_start(
                    xt[:, : XW // 2], xk[p, :, : XW // 2].bitcast(mm_dt)
                )
                nc.scalar.dma_start(
                    xt[:, XW // 2 :], xk[p, :, XW // 2 :].bitcast(mm_dt)
                )
                wt = wpool.tile([128, KC * E], mm_dt)
                weng = nc.sync if p % 2 == 0 else nc.scalar
                weng.dma_start(wt[:], wk[p].bitcast(mm_dt))
                ps = ppool.tile([E, B], mybir.dt.float32)
                for c in range(KC):
                    nc.tensor.matmul(
                        ps[:],
                        lhsT=wt[:, c * E : (c + 1) * E],
                        rhs=xt[:, c * B : (c + 1) * B],
                        start=(c == 0),
                        stop=(c == KC - 1),
                    )
                ot = opool.tile([E, B], mybir.dt.float32)
                nc.scalar.activation(
                    ot[:],
                    ps[:],
                    mybir.ActivationFunctionType.Identity,
                    bias=bt[:, p : p + 1],
                )
                nc.gpsimd.dma_start(y[p], ot[:])
    nc.compile()
    return nc


def _shard_inputs(x: np.ndarray, W: np.ndarray, b: np.ndarray):
    # xk[p, k, c*B + b] = x[b, 128c + k, p]
    xk = (
        x.transpose(2, 1, 0)
        .reshape(P, KC, 128, B)
        .transpose(0, 2, 1, 3)
        .reshape(P, 128, KC * B)
    )
    # wk[p, k, c*E + e] = W[p, 128c + k, e]
    wk = W.reshape(P, KC, 128, E).transpose(0, 2, 1, 3).reshape(P, 128, KC * E)
    bT = b.T  # [E, P]
    in_maps = []
    for m in range(NCORES):
        sl = slice(m * PL, (m + 1) * PL)
        in_maps.append(
            {
                "xk": np.ascontiguousarray(xk[sl]),
                "wk": np.ascontiguousarray(wk[sl]),
                "bT": np.ascontiguousarray(bT[:, sl]),
            }
        )
    return in_maps


def kernel(x: np.ndarray, W: np.ndarray, b: np.ndarray) -> np.ndarray:
    global _nc_cache, LAST_RESULTS
    x = np.ascontiguousarray(np.asarray(x, dtype=np.float32))
    W = np.ascontiguousarray(np.asarray(W, dtype=np.float32))
    b = np.ascontiguousarray(np.asarray(b, dtype=np.float32))
    if _nc_cache is None:
        _nc_cache = _build_nc()
    in_maps = _shard_inputs(x, W, b)
    res = run_bass_kernel_spmd(
        _nc_cache,
        in_maps,
        core_ids=list(range(NCORES)),
        trace=bool(os.environ.get("KERNEL_TRACE")),
    )
    LAST_RESULTS = res
    yall = np.concatenate([r["y"] for r in res.results], axis=0)  # [P, E, B]
    return np.ascontiguousarray(yall.transpose(2, 1, 0))  # [B, E, P]


# revision 9
# speedup vs baseline: 1.3336x; 1.2123x over previous
"""Trainium2 Bass kernel for nn_Loop_Projection (batched per-prototype GEMM).

Computes out[b, e, p] = sum_d x[b, d, p] * W[p, d, e] + b[p, e] with
x: [256, 512, 128] f32, W: [128, 512, 128] f32, b: [128, 128] f32.

Sharding: prototype axis P=128 split across 8 NeuronCores (16 protos each).
Each core's x/W slices are pre-transposed on the host so that every device
DMA is fully contiguous:
  xk[p][k, c*B + b] = x[b, 128c + k, p]      ([16, 128, 1024] per core)
  wk[p][k, c*E + e] = W[p, 128c + k, e]      ([16, 128, 512]  per core)
Per proto the kernel accumulates out.T = W_p.T @ x_p.T ([E, B] PSUM tile)
over 4 K-chunks of 128, adds the bias during the PSUM->SBUF copy, and
stores y[p] = [E, B] contiguous. The host reassembles [B, E, P].
"""

import os

import numpy as np

import concourse.bass as bass
import concourse.tile as tile
from concourse import bacc, mybir
from concourse.bass_utils import run_bass_kernel_spmd

B, D, P, E = 256, 512, 128, 128
NCORES = 8
PL = P // NCORES  # prototypes per core
KC = D // 128  # contraction chunks of 128

_nc_cache = None
LAST_RESULTS = None  # BassKernelResults of the most recent run (for test.py)

USE_FP32R = True  # float32r matmul: 1 cycle/row vs 4 for float32


def _build_nc() -> bass.Bass:
    nc = bacc.Bacc()
    xk = nc.dram_tensor("xk", [PL, 128, KC * B], mybir.dt.float32, kind="ExternalInput")
    wk = nc.dram_tensor("wk", [PL, 128, KC * E], mybir.dt.float32, kind="ExternalInput")
    bT = nc.dram_tensor("bT", [E, PL], mybir.dt.float32, kind="ExternalInput")
    y = nc.dram_tensor("y", [PL, E, B], mybir.dt.float32, kind="ExternalOutput")

    mm_dt = mybir.dt.float32r if USE_FP32R else mybir.dt.float32
    XW = KC * B  # 1024, x tile free width
    with tile.TileContext(nc) as tc:
        with (
            tc.tile_pool(name="const", bufs=1) as cpool,
            tc.tile_pool(name="xin", bufs=6) as xpool,
            tc.tile_pool(name="win", bufs=4) as wpool,
            tc.tile_pool(name="acc", bufs=4, space="PSUM") as ppool,
            tc.tile_pool(name="out", bufs=4) as opool,
        ):
            bt = cpool.tile([E, PL], mybir.dt.float32)
            nc.scalar.dma_start(bt[:], bT[:])
            for p in range(PL):
                # Split each x load across both HWDGE rings (SP + Act) and
                # alternate the W loads so both rings carry ~6 MiB; stores
                # ride the gpsimd SWDGE ring. One ring alone caps at ~260
                # GB/s, below the ~358 GB/s HBM-per-core limit.
                xt = xpool.tile([128, XW], mm_dt)
                nc.sync.dma_start(
                    xt[:, : XW // 2], xk[p, :, : XW // 2].bitcast(mm_dt)
                )
                nc.scalar.dma_start(
                    xt[:, XW // 2 :], xk[p, :, XW // 2 :].bitcast(mm_dt)
                )
                wt = wpool.tile([128, KC * E], mm_dt)
                weng = nc.sync if p % 2 == 0 else nc.scalar
                weng.dma_start(wt[:], wk[p].bitcast(mm_dt))
                ps = ppool.tile([E, B], mybir.dt.float32)
                for c in range(KC):
                    nc.tensor.matmul(
                        ps[:],
                        lhsT=wt[:, c * E : (c + 1) * E],
                        rhs=xt[:, c * B : (c + 1) * B],
                        start=(c == 0),
                        stop=(c == KC - 1),
                    )
                ot = opool.tile([E, B], mybir.dt.float32)
                # bias-add + PSUM->SBUF on the (otherwise idle) vector engine;
                # keeping it off scalar stops ACTIVATEs from stalling the Act
                # DMA ring's issue stream
                nc.vector.tensor_scalar_add(ot[:], ps[:], bt[:, p : p + 1])
                nc.gpsimd.dma_start(y[p], ot[:])
    nc.compile()
    return nc


def _shard_inputs(x: np.ndarray, W: np.ndarray, b: np.ndarray):
    # xk[p, k, c*B + b] = x[b, 128c + k, p]
    xk = (
        x.transpose(2, 1, 0)
        .reshape(P, KC, 128, B)
        .transpose(0, 2, 1, 3)
        .reshape(P, 128, KC * B)
    )
    # wk[p, k, c*E + e] = W[p, 128c + k, e]
    wk = W.reshape(P, KC, 128, E).transpose(0, 2, 1, 3).reshape(P, 128, KC * E)
    bT = b.T  # [E, P]
    in_maps = []
    for m in range(NCORES):
        sl = slice(m * PL, (m + 1) * PL)
        in_maps.append(
            {
                "xk": np.ascontiguousarray(xk[sl]),
                "wk": np.ascontiguousarray(wk[sl]),
                "bT": np.ascontiguousarray(bT[:, sl]),
            }
        )
    return in_maps


def kernel(x: np.ndarray, W: np.ndarray, b: np.ndarray) -> np.ndarray:
    global _nc_cache, LAST_RESULTS
    x = np.ascontiguousarray(np.asarray(x, dtype=np.float32))
    W = np.ascontiguousarray(np.asarray(W, dtype=np.float32))
    b = np.ascontiguousarray(np.asarray(b, dtype=np.float32))
    if _nc_cache is None:
        _nc_cache = _build_nc()
    in_maps = _shard_inputs(x, W, b)
    res = run_bass_kernel_spmd(
        _nc_cache,
        in_maps,
        core_ids=list(range(NCORES)),
        trace=bool(os.environ.get("KERNEL_TRACE")),
    )
    LAST_RESULTS = res
    yall = np.concatenate([r["y"] for r in res.results], axis=0)  # [P, E, B]
    return np.ascontiguousarray(yall.transpose(2, 1, 0))  # [B, E, P]
